# revision 10
# baseline (speedup 1.0000x reference)
"""CLUB loss kernel for Trainium2, 8-core data-parallel SPMD (v2).

Math: with flat_x (N,D) [from x (B,D,H,W) -> (B*H*W, D)], v = exp(-p_logvar),
  loss = (-0.5/N) * [ A - 2B - dot(m2, V) + 2*dot(m1, W) ]
where
  A  = sum_{i,d} x^2 v          B  = sum_{i,d} x mu v
  V_d = sum_i v                 W_d = sum_i mu v
  m1 = S1/N, m2 = S2/N,  S1_d = sum_i x,  S2_d = sum_i x^2
All terms are per-core-local partial sums; the tiny (~KB) cross-core
reduction and final dot products happen on host in float64.

Layout: d-major (partition = d); mu/lv are transposed on PE via identity
matmuls (128x128 blocks into PSUM). All reductions ride accum_out on ops
we need anyway. Engine split keeps every engine under the ~35us DMA
window: ACT = exp (half-units, early ramp) + x^2 (full units); DVE =
w = muT*v (halves) + a = x2*v + b(b1) = w*x; GPSIMD = S1 pass
((x*-1)-x -> -2x, accum) + b(b0) + the x DMA issues. mu/lv stream on the
Sync HWDGE queue as 0.5-1 MiB slabs, x on the gpsimd queue; the last x
quarters are small so the post-last-DMA tail is a short sq->a->b chain.
"""

import sys

import numpy as np

for _p in ("/opt/trn_rl_repo",):
    if _p not in sys.path:
        sys.path.append(_p)

B, D, H, W = 16, 512, 32, 32
HW = H * W
N = B * HW
NCORES = 8
BLKB = B // NCORES          # b-blocks per core (2)
ROWS = N // NCORES          # rows per core (2048)
NDC = D // 128              # d chunks (4)
NU = BLKB * NDC             # full units per core (8)
HHW = HW // 2               # i-extent of a half-unit (512)

_prog_cache = {}


def build_program():
    import concourse.bacc as bacc
    import concourse.tile as tile
    from concourse import mybir

    f32 = mybir.dt.float32
    AF = mybir.ActivationFunctionType
    OP = mybir.AluOpType

    nc = bacc.Bacc(
        "TRN2",
        target_bir_lowering=False,
        debug=False,
        enable_asserts=False,
        num_devices=NCORES,
    )

    x_d = nc.dram_tensor("x_s", (BLKB, D, HW), f32, kind="ExternalInput").ap()
    mu_d = nc.dram_tensor("mu_s", (ROWS, D), f32, kind="ExternalInput").ap()
    lv_d = nc.dram_tensor("lv_s", (ROWS, D), f32, kind="ExternalInput").ap()
    id_d = nc.dram_tensor("ident", (128, 128), f32, kind="ExternalInput").ap()

    # acc columns: [0:16) V per half | [16:32) W per half | [32:40) A per unit
    # [40:48) B | [48:56) S1 | [56:64) S2     (unit u = b*NDC+dc, half
    # col = 2u+h; partition p -> d = 128*dc + p)
    o_misc = nc.dram_tensor("o_misc", (128, 64), f32, kind="ExternalOutput").ap()

    with tile.TileContext(nc) as tc:
        with (
            tc.tile_pool(name="const", bufs=1) as constp,
            tc.tile_pool(name="slab", bufs=4) as slp,
            tc.tile_pool(name="xpool", bufs=2) as xp,
            tc.tile_pool(name="vw", bufs=5) as vwp,
            tc.tile_pool(name="x2p", bufs=5) as x2p,
            tc.tile_pool(name="scr", bufs=2) as scrp,
            tc.tile_pool(name="accum", bufs=1) as accp,
            tc.tile_pool(name="psum", bufs=3, space="PSUM") as pp,
        ):
            ident = constp.tile([128, 128], f32)
            acc = accp.tile([128, 64], f32, tag="acc", name="acc")

            lv_slabs = {}
            mu_slabs = {}
            xq = {}

            # mu/lv slab s covers rows [512s, 512(s+1)) = i-tiles [4s, 4s+4)
            # = the (b, h) half-block with s = 2b+h.  [128, 4, 512]:
            # partition p holds rows {128g+p : g}, 2 KiB lines.
            def load_slab(dram, store, s, tag, part=None):
                t_ = store.get(s)
                if t_ is None:
                    t_ = slp.tile([128, 4 * D], f32, tag=tag, name=tag)
                    store[s] = t_
                rows = dram[512 * s : 512 * (s + 1), :]
                src = rows.rearrange("(g p) f -> p g f", p=128)
                if part is None:
                    nc.sync.dma_start(t_[:], src)
                else:
                    # half-slab load (g in [2*part, 2*part+2)), 512 KiB
                    nc.sync.dma_start(
                        t_[:, 2 * part * D : (2 * part + 2) * D],
                        rows[256 * part : 256 * (part + 1), :].rearrange(
                            "(g p) f -> p g f", p=128
                        ),
                    )

            # x block b: [128, 4, 1024]: partition p holds rows {128c+p : c}
            # (i.e. d = 128c+p -> free block c), 4 KiB lines.  Loaded on the
            # gpsimd (SWDGE) queue so it streams concurrently with mu/lv.
            def load_x(b, c0, c1):
                t_ = xq.get(b)
                if t_ is None:
                    t_ = xp.tile([128, 4 * HW], f32, tag="x", name="x")
                    xq[b] = t_
                rows = x_d[b, 128 * c0 : 128 * c1, :]
                nc.gpsimd.dma_start(
                    t_[:, c0 * HW : c1 * HW],
                    rows.rearrange("(c p) f -> p c f", p=128),
                )

            # ---- DMA issue: mu/lv stream (sync queue), x stream (gpsimd) --
            # first slabs in small pieces so the transpose->exp->w ramp
            # starts as early as possible
            load_slab(lv_d, lv_slabs, 0, "lv_sl", part=0)
            nc.sync.dma_start(ident[:], id_d[:])
            load_slab(mu_d, mu_slabs, 0, "mu_sl", part=0)
            load_slab(lv_d, lv_slabs, 0, "lv_sl", part=1)
            load_slab(mu_d, mu_slabs, 0, "mu_sl", part=1)
            load_slab(lv_d, lv_slabs, 1, "lv_sl")
            load_slab(mu_d, mu_slabs, 1, "mu_sl")
            load_slab(lv_d, lv_slabs, 2, "lv_sl")
            load_slab(mu_d, mu_slabs, 2, "mu_sl")
            load_slab(lv_d, lv_slabs, 3, "lv_sl")
            load_slab(mu_d, mu_slabs, 3, "mu_sl")

            load_x(0, 0, 1)       # 512 KiB (early sq/S1 start)
            load_x(0, 1, 4)       # 1.5 MiB
            load_x(1, 0, 2)       # 1 MiB
            load_x(1, 2, 3)       # 512 KiB
            load_x(1, 3, 4)       # 512 KiB (last x; short sq->a->b tail)

            v_u = {}
            w_u = {}
            x2_u = {}

            def transpose_half(store, b, h, dc, tag):
                s = 2 * b + h
                t_ = pp.tile([128, HHW], f32, tag=tag, name=tag)
                for g in range(4):
                    nc.tensor.matmul(
                        t_[:, 128 * g : 128 * (g + 1)],
                        store[s][:, D * g + 128 * dc : D * g + 128 * dc + 128],
                        ident[:],
                        is_transpose=True,
                        start=(g == 0),
                        stop=(g == 3),
                    )
                return t_

            # ---- compute waves ------------------------------------------
            for b in range(BLKB):
                for h in range(2):
                    for dc in range(NDC):
                        u = b * NDC + dc
                        hc = 2 * u + h

                        lvT = transpose_half(lv_slabs, b, h, dc, "lvT")
                        muT = transpose_half(mu_slabs, b, h, dc, "muT")

                        if h == 0:
                            v_u[u] = vwp.tile([128, HW], f32, tag="v", name="v")
                            w_u[u] = vwp.tile([128, HW], f32, tag="w", name="w")
                        # exp (ACT): v half, accum V
                        nc.scalar.activation(
                            v_u[u][:, HHW * h : HHW * (h + 1)],
                            lvT[:], AF.Exp, scale=-1.0,
                            accum_out=acc[:, hc : hc + 1],
                        )
                        # w = muT * v (DVE, PSUM operand), accum W
                        nc.vector.scalar_tensor_tensor(
                            out=w_u[u][:, HHW * h : HHW * (h + 1)],
                            in0=muT[:], scalar=1.0,
                            in1=v_u[u][:, HHW * h : HHW * (h + 1)],
                            op0=OP.mult, op1=OP.mult,
                            accum_out=acc[:, 16 + hc : 16 + hc + 1],
                        )

                    if h == 0:
                        # x-only passes for this b (full units): sq on ACT
                        # (x2 tensor + S2 accum); S1 = sum_i x via ACT copy
                        # (b=0) / DVE tensor_scalar at 2x (b=1)
                        for dc in range(NDC):
                            u = b * NDC + dc
                            xs = xq[b][:, HW * dc : HW * (dc + 1)]
                            x2_u[u] = x2p.tile(
                                [128, HW], f32, tag="x2", name="x2"
                            )
                            nc.scalar.activation(
                                x2_u[u][:], xs, AF.Square,
                                accum_out=acc[:, 56 + u : 56 + u + 1],
                            )
                            s1scr = scrp.tile(
                                [128, HW], f32, tag="s1scr", name="s1scr"
                            )
                            if b == 0:
                                nc.scalar.activation(
                                    s1scr[:], xs, AF.Copy,
                                    accum_out=acc[:, 48 + u : 48 + u + 1],
                                )
                            else:
                                nc.vector.tensor_scalar(
                                    s1scr[:], xs, 1.0, 0.0, OP.mult, OP.add,
                                    accum_out=acc[:, 48 + u : 48 + u + 1],
                                )
                    else:
                        # full-unit passes needing v/w complete: a, b on DVE
                        for dc in range(NDC):
                            u = b * NDC + dc
                            xs = xq[b][:, HW * dc : HW * (dc + 1)]
                            a_scr = scrp.tile(
                                [128, HW], f32, tag="ascr", name="ascr"
                            )
                            nc.vector.scalar_tensor_tensor(
                                out=a_scr[:], in0=x2_u[u][:], scalar=1.0,
                                in1=v_u[u][:],
                                op0=OP.mult, op1=OP.mult,
                                accum_out=acc[:, 32 + u : 32 + u + 1],
                            )
                            b_scr = scrp.tile(
                                [128, HW], f32, tag="bscr", name="bscr"
                            )
                            nc.vector.scalar_tensor_tensor(
                                out=b_scr[:], in0=w_u[u][:], scalar=1.0,
                                in1=xs,
                                op0=OP.mult, op1=OP.mult,
                                accum_out=acc[:, 40 + u : 40 + u + 1],
                            )

            nc.sync.dma_start(o_misc[:], acc[:])

    nc.compile()
    return nc


def get_program():
    if "nc" not in _prog_cache:
        _prog_cache["nc"] = build_program()
    return _prog_cache["nc"]


def make_in_maps(x, p_mu, p_logvar):
    x = np.ascontiguousarray(np.asarray(x, dtype=np.float32)).reshape(B, D, HW)
    p_mu = np.ascontiguousarray(np.asarray(p_mu, dtype=np.float32))
    p_logvar = np.ascontiguousarray(np.asarray(p_logvar, dtype=np.float32))
    in_maps = []
    for c in range(NCORES):
        in_maps.append(
            {
                "x_s": np.ascontiguousarray(x[BLKB * c : BLKB * (c + 1)]),
                "mu_s": np.ascontiguousarray(p_mu[ROWS * c : ROWS * (c + 1)]),
                "lv_s": np.ascontiguousarray(p_logvar[ROWS * c : ROWS * (c + 1)]),
                "ident": np.eye(128, dtype=np.float32),
            }
        )
    return in_maps


def finish_host(results):
    """Combine per-core partials (float64) into the scalar loss."""
    Vv = np.zeros(D)
    Ww = np.zeros(D)
    S1 = np.zeros(D)
    S2 = np.zeros(D)
    A = 0.0
    Bb = 0.0
    for r in results:
        misc = r["o_misc"].astype(np.float64)
        for u in range(NU):
            dc = u % NDC
            dsl = slice(128 * dc, 128 * (dc + 1))
            Vv[dsl] += misc[:, 2 * u] + misc[:, 2 * u + 1]
            Ww[dsl] += misc[:, 16 + 2 * u] + misc[:, 16 + 2 * u + 1]
            A += float(misc[:, 32 + u].sum())
            Bb += float(misc[:, 40 + u].sum())
            S1[dsl] += misc[:, 48 + u]
            S2[dsl] += misc[:, 56 + u]
    m1 = S1 / N
    m2 = S2 / N
    S = A - 2.0 * Bb - float(np.dot(m2, Vv)) + 2.0 * float(np.dot(m1, Ww))
    return np.float32(-0.5 / N * S)


def run_on_device(x, p_mu, p_logvar, trace=False, **kw):
    from concourse import bass_utils

    nc = get_program()
    in_maps = make_in_maps(x, p_mu, p_logvar)
    return bass_utils.run_bass_kernel_spmd(
        nc, in_maps, list(range(NCORES)), trace=trace, **kw
    )


def kernel(x, p_mu, p_logvar):
    res = run_on_device(x, p_mu, p_logvar)
    return finish_host(res.results)
